# revision 40
# baseline (speedup 1.0000x reference)
"""Causal self-attention (B=4, T=4096, D=768, single head, fp32 in/out) on 8
TRN2 NeuronCores.

Sharding: core <-> (batch b = core//2, parity h = core%2). Each core handles
the 16 query tiles (128 rows) at global tile index g = 2i + h for local
i = 0..15 (parity interleave balances causal work across the pair to ~3%).
Per local q-tile i the kernel computes scores against keys [0, 256*(i+1)):
columns below 256*i are always causally allowed for both parities; the last
256 columns are fixed up with per-core input mask tiles.

Math restructure vs a direct implementation — no Q/K/V projections at all:
  scores:  S^T = K.Q^T = x_kv . (W_q^T W_k) . x_q^T. M2 = W_q^T W_k is
           computed once (768x768, 11.5us) and R = M2^T x_q^T once
           (768x2048); per kv superblock S^T = x_kv . R uses the streamed
           x^T tile directly as the stationary operand.
  output:  O = P^T V = P^T x_kv W_v^T re-associates to (x_kv^T P^T)^T-style:
           the kernel accumulates G^T[d, q] = sum_k x_kv[k, d] P[k, q]
           across all kv (stationary = x_kv in [kv, d] layout, moving = the
           P^T tile that exp already produces), then applies one output
           transform O[q,:] = sum_d G^T[d, q] W_v^T[d, :] per retired
           q-tile. This replaces the 61us V projection (which was also
           fully duplicated across the core pair) with a 31us transform
           over query columns only.
  softmax: the denominator l[q] = sum_k P[k, q] comes from a per-q-tile
           ones-matmul (K=128, N=1) over a DVE-presummed P (the 4 kv tiles
           of a superblock added elementwise), landing directly in
           [q-partition, 1] form for the final per-partition 1/l scale.
           No max-subtraction pass: scores are ~N(0,1) so exp cannot
           overflow.

All matmul operands are fp16 (PSUM accumulation fp32): fp16 enables the
compiler's fast-weight-load path (fp32 stationary loads at ~190 ns dominate
the PE pipe otherwise) and halves DMA. Output returns as fp16, upcast on
host. Max relative error vs the fp32 reference ~1e-3, vs the 2e-2 gate.

PSUM (8 banks): S^T pool 3 x [128,512] | G^T wave pool 4 x [128,512] |
1 shared bank for l / output-transform. G^T accumulates over a superblock's
4 kv tiles in 3 waves of 2 d-chunks, drained by DVE adds into a [128,
6x2048] fp32 SBUF accumulator.
"""

import os
import sys
from contextlib import ExitStack

import numpy as np

if "/opt/trn_rl_repo" not in sys.path:
    sys.path.insert(0, "/opt/trn_rl_repo")

B, T, D = 4, 4096, 768
N_CORES = 8
QTILES = 16          # local q-tiles per core, 128 rows each
EC = D // 128        # 6 d chunks of 128
SB = 8               # kv superblocks
SBW = 512            # superblock width (keys)
NKT = SBW // 128     # kv 128-tiles per superblock
QW = QTILES * 128    # query columns per core
NEG = -1.0e9
SCALE = 1.0 / float(np.sqrt(D))

_CACHE = {}


def _patch_tile_drain():
    """This walrus build accepts only one sync wait per instruction;
    TileContext's tail drain carries one wait per outstanding proc. Split
    them onto individual SP no-ops (SP executes sequentially, so semantics
    are unchanged)."""
    import concourse.mybir as mybir
    import concourse.tile as tile
    from concourse.vector_clock import ScopedClock

    if getattr(tile.TileContext, "_drain_split_patch", False):
        return

    def _split_drain_and_barrier(self, tick_clock, wait_clock):
        nc = self.nc
        carrier = nc.sync.nop(nofuse=True)
        wait_clock.add_sem_waits(
            carrier.ins, ScopedClock({None: tick_clock.global_clock})
        )
        si = carrier.ins.sync_info
        waits = list(si.on_wait) if si is not None else []
        carrier.ins.sync_info = mybir.SyncInfo(on_wait=waits[:1], on_update=[])
        for w in waits[1:]:
            n = nc.sync.nop(nofuse=True)
            n.ins.sync_info = mybir.SyncInfo(on_wait=[w], on_update=[])
        nc.sync.drain()
        nc.all_engine_barrier()
        assert self.sems is not None
        popped = nc._tile_sem_poison_stack.pop()
        assert popped is self._sem_poison
        nc.clear_and_free_semaphores(list(self.sems.allocated().values()))
        nc.all_engine_barrier()

    tile.TileContext._drain_and_barrier = _split_drain_and_barrier
    tile.TileContext._drain_split_patch = True


def _hoist_multi_waits(nc):
    """This walrus build encodes at most ONE sync wait per instruction
    descriptor. Tile's sem assignment can put several waits on one
    instruction; hoist the extras onto same-engine no-ops inserted
    immediately before it — the engine executes them sequentially, so the
    wait semantics are unchanged."""
    import concourse.mybir as mybir

    n = 0
    for fn in nc.m.functions:
        for bb in fn.blocks:
            insts = bb.instructions
            out = []
            for ins in insts:
                si = ins.sync_info
                waits = list(si.on_wait) if si is not None else []
                if len(waits) > 1:
                    for w in waits[:-1]:
                        nop = mybir.InstNoOp(
                            name=f"I-hoistw-{nc.next_id()}",
                            engine=ins.engine,
                            ins=[],
                            outs=[],
                            sync_info=mybir.SyncInfo(on_wait=[w], on_update=[]),
                        )
                        out.append(nop)
                        n += 1
                    ins.sync_info = mybir.SyncInfo(
                        on_wait=[waits[-1]], on_update=list(si.on_update)
                    )
                out.append(ins)
            insts[:] = out
    return n


def _build_program(hoist=True):
    import concourse.bass as bass
    import concourse.mybir as mybir
    import concourse.tile as tile

    _patch_tile_drain()
    f32 = mybir.dt.float32
    f16 = mybir.dt.float16
    Exp = mybir.ActivationFunctionType.Exp
    Copy = mybir.ActivationFunctionType.Copy

    nc = bass.Bass()
    # x^T per core batch: [d_part, d_chunk, kv] — stationary operand of S^T
    xkvT = nc.dram_tensor("xkvT", [128, EC, T], f16, kind="ExternalInput")
    # x rows per core batch: [kv_part, kv_tile, d] — stationary operand of G^T
    xkvK = nc.dram_tensor("xkvK", [128, SB * NKT, D], f16, kind="ExternalInput")
    xqT = nc.dram_tensor("xqT", [128, EC, QW], f16, kind="ExternalInput")
    # W_q, W_k in stored [e, d] layout chunked over e; W_v^T chunked over d
    wqE = nc.dram_tensor("wqE", [128, EC, D], f16, kind="ExternalInput")
    wkE = nc.dram_tensor("wkE", [128, EC, D], f16, kind="ExternalInput")
    wvT = nc.dram_tensor("wvT", [128, EC, D], f16, kind="ExternalInput")
    # masks are [kv, q] (transposed) here; maskf is all -1e9
    maska = nc.dram_tensor("maska", [128, 128], f32, kind="ExternalInput")
    maskb = nc.dram_tensor("maskb", [128, 128], f32, kind="ExternalInput")
    maskf = nc.dram_tensor("maskf", [128, 128], f32, kind="ExternalInput")
    out_d = nc.dram_tensor("out", [QW, D], f16, kind="ExternalOutput")

    with tile.TileContext(nc) as tc:
        with (
            tc.tile_pool(name="consts", bufs=1) as cpool,
            tc.tile_pool(name="wk", bufs=1) as wkpool,
            tc.tile_pool(name="rt", bufs=1) as rtpool,
            tc.tile_pool(name="gacc", bufs=1) as gapool,
            tc.tile_pool(name="ps_st", bufs=3, space="PSUM") as ps_st,
            tc.tile_pool(name="ps_gt", bufs=4, space="PSUM") as ps_gt,
            tc.tile_pool(name="ps_lo", bufs=1, space="PSUM") as ps_lo,
        ):
            # PE warm-up: the HAM clock gate starts at 1.2 GHz and needs
            # ~3.4us of sustained matmul activity to release to 2.4 GHz.
            # Burn the initial DMA wait on dummy matmuls.
            warm_t = cpool.tile([128, 512], f16, tag="warm")
            nc.vector.memset(warm_t[:], 0.0)
            for w in range(26):
                wps = ps_gt.tile([128, 512], f32, tag="gt", name=f"warm{w}")
                nc.tensor.matmul(
                    wps[:], warm_t[:, :128], warm_t[:], start=True, stop=True
                )

            # x superblock tiles prefetch in dedicated pools (outside the
            # transient qproj space) so sb0/sb1 stream during M2/R.
            xtpool_cm = tc.tile_pool(name="xt", bufs=2)
            xtpool = xtpool_cm.__enter__()
            xkpool_cm = tc.tile_pool(name="xk", bufs=2)
            xkpool = xkpool_cm.__enter__()
            qppool_cm = tc.tile_pool(name="qproj", bufs=1)
            qppool = qppool_cm.__enter__()
            wq_t = qppool.tile([128, EC, D], f16, tag="wq")
            nc.sync.dma_start(out=wq_t[:], in_=wqE[:])
            wk_t = qppool.tile([128, EC, D], f16, tag="wk")
            nc.sync.dma_start(out=wk_t[:], in_=wkE[:])
            xq_t = qppool.tile([128, EC, QW], f16, tag="xq")
            nc.sync.dma_start(out=xq_t[:], in_=xqT[:])
            m2_t = qppool.tile([128, EC, D], f16, tag="m2")
            xT_tiles = {}
            xK_tiles = {}
            for sb in range(2):
                xT_tiles[sb] = xtpool.tile(
                    [128, EC, SBW], f16, tag="xt", name=f"xT{sb}"
                )
                nc.sync.dma_start(
                    out=xT_tiles[sb][:],
                    in_=xkvT[:, :, sb * SBW : (sb + 1) * SBW],
                )
                xK_tiles[sb] = xkpool.tile(
                    [128, NKT, D], f16, tag="xk", name=f"xK{sb}"
                )
                nc.sync.dma_start(
                    out=xK_tiles[sb][:],
                    in_=xkvK[:, sb * NKT : (sb + 1) * NKT, :],
                )
            ma_t = cpool.tile([128, 128], f32, tag="ma")
            nc.sync.dma_start(out=ma_t[:], in_=maska[:])
            mb_t = cpool.tile([128, 128], f32, tag="mb")
            nc.sync.dma_start(out=mb_t[:], in_=maskb[:])
            mf_t = cpool.tile([128, 128], f32, tag="mf")
            nc.sync.dma_start(out=mf_t[:], in_=maskf[:])
            wv_t = wkpool.tile([128, EC, D], f16, tag="wv")
            nc.sync.dma_start(out=wv_t[:], in_=wvT[:])
            ones_t = cpool.tile([128, 1], f16, tag="ones")
            nc.vector.memset(ones_t[:], 1.0)

            # ---- M2 = W_q^T W_k  [d2, d], chunked over d2 ----
            for a in range(EC):
                ps = ps_gt.tile([128, 512], f32, tag="gt", name=f"psm2a{a}")
                ps2 = ps_gt.tile([128, 512], f32, tag="gt", name=f"psm2b{a}")
                for j in range(EC):
                    nc.tensor.matmul(
                        ps[:],
                        wq_t[:, j, a * 128 : (a + 1) * 128],
                        wk_t[:, j, 0:512],
                        start=(j == 0),
                        stop=(j == EC - 1),
                    )
                    nc.tensor.matmul(
                        ps2[:, :256],
                        wq_t[:, j, a * 128 : (a + 1) * 128],
                        wk_t[:, j, 512:768],
                        start=(j == 0),
                        stop=(j == EC - 1),
                    )
                nc.scalar.copy(out=m2_t[:, a, 0:512], in_=ps[:])
                nc.scalar.copy(out=m2_t[:, a, 512:768], in_=ps2[:, :256])

            # ---- R = M2^T x_q^T  [d, q] resident in SBUF ----
            r_t = rtpool.tile([128, EC, QW], f16, tag="rt")
            for m in range(EC):
                pss = [
                    ps_st.tile([128, 512], f32, tag="st", name=f"psr{m}_{qc}")
                    for qc in range(3)
                ] + [ps_gt.tile([128, 512], f32, tag="gt", name=f"psr3_{m}")]
                for j in range(EC):
                    for qc in range(4):
                        nc.tensor.matmul(
                            pss[qc][:],
                            m2_t[:, j, m * 128 : (m + 1) * 128],
                            xq_t[:, j, qc * 512 : (qc + 1) * 512],
                            start=(j == 0),
                            stop=(j == EC - 1),
                        )
                for qc in range(4):
                    nc.scalar.copy(
                        out=r_t[:, m, qc * 512 : (qc + 1) * 512], in_=pss[qc][:]
                    )
            qppool_cm.__exit__(None, None, None)

            # G^T accumulator [d_part, d_chunk, q] fp32 and l accumulator
            gacc_t = gapool.tile([128, EC, QW], f32, tag="gacc")
            lacc_t = gapool.tile([128, QTILES], f32, tag="lacc")

            # ---- kv superblocks ----
            attn_pools = ExitStack()
            ptpool = attn_pools.enter_context(tc.tile_pool(name="pt", bufs=2))
            p4pool = attn_pools.enter_context(tc.tile_pool(name="p4", bufs=2))
            gqpool = attn_pools.enter_context(tc.tile_pool(name="gq", bufs=2))
            spool = attn_pools.enter_context(tc.tile_pool(name="small", bufs=2))
            obpool = attn_pools.enter_context(tc.tile_pool(name="ob", bufs=2))
            for sb in range(SB):
                xT_t = xT_tiles.pop(sb)
                xK_t = xK_tiles.pop(sb)
                if sb + 2 < SB:  # prefetch two superblocks ahead
                    xT_tiles[sb + 2] = xtpool.tile(
                        [128, EC, SBW], f16, tag="xt", name=f"xT{sb + 2}"
                    )
                    nc.sync.dma_start(
                        out=xT_tiles[sb + 2][:],
                        in_=xkvT[:, :, (sb + 2) * SBW : (sb + 3) * SBW],
                    )
                    xK_tiles[sb + 2] = xkpool.tile(
                        [128, NKT, D], f16, tag="xk", name=f"xK{sb + 2}"
                    )
                    nc.sync.dma_start(
                        out=xK_tiles[sb + 2][:],
                        in_=xkvK[:, (sb + 2) * NKT : (sb + 3) * NKT, :],
                    )

                # ---- attention, in q-groups of up to 512 columns ----
                # active q-tiles: i in [2*sb, 16); groups are 512-aligned
                i_lo = 2 * sb
                g_lo = i_lo // 4
                for g in range(g_lo, 4):
                    ia = max(i_lo, 4 * g)      # first active q-tile in group
                    ib = 4 * g + 4             # end q-tile (exclusive)
                    qc0 = ia * 128             # first active q column
                    gw = (ib - ia) * 128       # group width (256 or 512)

                    # S^T = x_kv . R for the group's q span, per kv-tile.
                    # In the sb's first group, q-tile 2sb sits at position
                    # 0 and its kv-tiles 2,3 are fully masked on BOTH
                    # parities (SPMD-safe intersection): skip those 128
                    # columns entirely — their P is memset to 0 instead.
                    narrow = {2: 128, 3: 128} if g == g_lo else {}
                    stg = [
                        ps_st.tile([128, 512], f32, tag="st", name=f"st{sb}_{g}_{k}")
                        for k in range(NKT)
                    ]
                    for kt in range(NKT):
                        nlo = narrow.get(kt, 0)
                        for j in range(EC):
                            nc.tensor.matmul(
                                stg[kt][:, nlo:gw],
                                xT_t[:, j, kt * 128 : (kt + 1) * 128],
                                r_t[:, j, qc0 + nlo : qc0 + gw],
                                start=(j == 0),
                                stop=(j == EC - 1),
                            )
                    # causal fixups for the terminal q-tiles of this sb:
                    # q-tile 2sb terminates at kv-tiles (0,1) of this sb
                    # (mask a,b; tiles 2,3 fully masked); q-tile 2sb+1
                    # terminates at kv-tiles (2,3).
                    for i, kts in ((2 * sb, (ma_t, mb_t, mf_t, mf_t)),
                                   (2 * sb + 1, (None, None, ma_t, mb_t))):
                        if not (ia <= i < ib):
                            continue
                        qo = i * 128 - qc0
                        for kt in range(NKT):
                            m = kts[kt]
                            if m is not None and qo >= narrow.get(kt, 0):
                                nc.vector.tensor_add(
                                    stg[kt][:, qo : qo + 128],
                                    stg[kt][:, qo : qo + 128],
                                    m[:],
                                )
                    # P^T = exp(S^T * scale) back to SBUF (fp16); narrowed
                    # columns never touch PSUM — memset their P to 0.
                    pt_t = ptpool.tile([128, NKT, 512], f16, tag="pt")
                    for kt in range(NKT):
                        nlo = narrow.get(kt, 0)
                        if nlo:
                            nc.vector.memset(pt_t[:, kt, :nlo], 0.0)
                        nc.scalar.activation(
                            pt_t[:, kt, nlo:gw], stg[kt][:, nlo:gw], Exp, scale=SCALE
                        )
                    # l path: DVE-presum the 4 P^T kv tiles elementwise
                    # (fp16, on the slack Vector engine); then ONE
                    # K=128/N=1 ones-matmul per active q-tile lands the
                    # denominator directly in [q_part, 1] form.
                    p4_t = p4pool.tile([128, 512], f16, tag="p4")
                    nc.vector.tensor_add(
                        p4_t[:, :gw], pt_t[:, 0, :gw], pt_t[:, 1, :gw]
                    )
                    nc.vector.tensor_add(
                        p4_t[:, :gw], p4_t[:, :gw], pt_t[:, 2, :gw]
                    )
                    nc.vector.tensor_add(
                        p4_t[:, :gw], p4_t[:, :gw], pt_t[:, 3, :gw]
                    )
                    # G^T += x_kv^T . P^T in 3 waves of 2 d-chunks. The l
                    # ones-matmuls (array-cheap but LDW-serial in the
                    # in-order PE queue) interleave between waves so their
                    # weight loads hide under the waves' streaming.
                    lq = list(range(ia, ib))
                    for wv in range(3):
                        gps = [
                            ps_gt.tile(
                                [128, 512], f32, tag="gt",
                                name=f"gt{sb}_{g}_{wv}_{c}",
                            )
                            for c in range(2)
                        ]
                        for kt in range(NKT):
                            nlo = narrow.get(kt, 0)
                            for c in range(2):
                                mch = 2 * wv + c
                                nc.tensor.matmul(
                                    gps[c][:, nlo:gw],
                                    xK_t[:, kt, mch * 128 : (mch + 1) * 128],
                                    pt_t[:, kt, nlo:gw],
                                    start=(kt == 0),
                                    stop=(kt == NKT - 1),
                                )
                        for _ in range(2 if wv == 2 else 1):
                            if not lq:
                                continue
                            i = lq.pop(0)
                            qo = i * 128 - qc0
                            lps = ps_lo.tile(
                                [128, 512], f32, tag="lo", name=f"l{sb}_{i}"
                            )
                            nc.tensor.matmul(
                                lps[:, 0:1],
                                p4_t[:, qo : qo + 128],
                                ones_t[:],
                                start=True,
                                stop=True,
                            )
                            lsl = lacc_t[:, i : i + 1]
                            if sb == 0:
                                nc.vector.tensor_copy(out=lsl, in_=lps[:, 0:1])
                            else:
                                nc.vector.tensor_add(lsl, lps[:, 0:1], lsl)
                        for c in range(2):
                            mch = 2 * wv + c
                            gsl = gacc_t[:, mch, qc0 : qc0 + gw]
                            if sb == 0:
                                nc.vector.tensor_copy(out=gsl, in_=gps[c][:, :gw])
                            else:
                                nc.vector.tensor_add(gsl, gps[c][:, :gw], gsl)

                    # ---- retire terminal q-tiles: O = G^T^T . W_v^T ----
                    for i in range(ia, ib):
                        if i not in (2 * sb, 2 * sb + 1):
                            continue
                        # convert this q-tile's G^T slice to fp16 stationary
                        gq_t = gqpool.tile([128, EC, 128], f16, tag="gq")
                        nc.scalar.copy(
                            out=gq_t[:], in_=gacc_t[:, :, i * 128 : (i + 1) * 128]
                        )
                        recip = spool.tile([128, 1], f32, tag="recip")
                        nc.vector.reciprocal(out=recip[:], in_=lacc_t[:, i : i + 1])
                        ob = obpool.tile([128, D], f16, tag="ob")
                        for lo, n in ((0, 512), (512, 256)):
                            # second half borrows a Gt-pool bank so the two
                            # matmul groups + normalizes pipeline
                            pool = ps_lo if lo == 0 else ps_gt
                            ops = pool.tile(
                                [128, 512], f32,
                                tag=("lo" if lo == 0 else "gt"),
                                name=f"o{sb}_{i}_{lo}",
                            )
                            for m in range(EC):
                                nc.tensor.matmul(
                                    ops[:, :n],
                                    gq_t[:, m, :],
                                    wv_t[:, m, lo : lo + n],
                                    start=(m == 0),
                                    stop=(m == EC - 1),
                                )
                            nc.scalar.activation(
                                ob[:, lo : lo + n],
                                ops[:, :n],
                                Copy,
                                scale=recip[:, 0:1],
                            )
                        nc.sync.dma_start(
                            out=out_d[i * 128 : (i + 1) * 128, :], in_=ob[:]
                        )
            attn_pools.close()
            xkpool_cm.__exit__(None, None, None)
            xtpool_cm.__exit__(None, None, None)
    if hoist:
        _hoist_multi_waits(nc)
    return nc


def _prep_inputs(x, W_q, W_k, W_v):
    """Per-core input maps. Host-side work is layout + fp16 cast only."""

    def chunked(a):  # [768, N] -> [128, EC, N]
        return np.ascontiguousarray(
            a.reshape(EC, 128, -1).transpose(1, 0, 2).astype(np.float16)
        )

    wqE = chunked(W_q)           # [e, d] chunked over e
    wkE = chunked(W_k)
    wvT = chunked(W_v.T.copy())  # [d, e] chunked over d

    r = np.arange(128, dtype=np.float32)
    # [q, c] triangle: allowed iff c <= q; stored transposed ([kv, q])
    tri = np.where(r[None, :] <= r[:, None], 0.0, NEG).astype(np.float32)
    triT = np.ascontiguousarray(tri.T)
    zero = np.zeros((128, 128), dtype=np.float32)
    full = np.full((128, 128), NEG, dtype=np.float32)
    # per-parity (maska, maskb) for the terminal 256 kv columns
    masks_ab = [(triT, full), (zero, triT)]

    in_maps = []
    qsels = []
    for c in range(N_CORES):
        b, h = c // 2, c % 2
        xT = chunked(np.ascontiguousarray(x[b].T))  # [128, EC, T] fp16
        # [kv_part, kv_tile, d]: row sb*512 + kt*128 + p -> [p, sb*4+kt, :]
        xK = np.ascontiguousarray(
            x[b].reshape(SB * NKT, 128, D).transpose(1, 0, 2).astype(np.float16)
        )
        qsel = np.concatenate(
            [np.arange((2 * i + h) * 128, (2 * i + h + 1) * 128) for i in range(QTILES)]
        )
        qsels.append(qsel)
        ma, mb = masks_ab[h]
        in_maps.append(
            {
                "xkvT": xT,
                "xkvK": xK,
                "xqT": np.ascontiguousarray(xT[:, :, qsel]),
                "wqE": wqE,
                "wkE": wkE,
                "wvT": wvT,
                "maska": ma,
                "maskb": mb,
                "maskf": full,
            }
        )
    return in_maps, qsels


def kernel(x, W_q, W_k, W_v, _trace=False):
    from concourse.bass_utils import run_bass_kernel_spmd

    if "nc" not in _CACHE:
        _CACHE["nc"] = _build_program()
    nc = _CACHE["nc"]

    in_maps, qsels = _prep_inputs(
        np.asarray(x, dtype=np.float32),
        np.asarray(W_q, dtype=np.float32),
        np.asarray(W_k, dtype=np.float32),
        np.asarray(W_v, dtype=np.float32),
    )
    res = run_bass_kernel_spmd(nc, in_maps, list(range(N_CORES)), trace=_trace)
    _CACHE["last_results"] = res

    out = np.empty((B, T, D), dtype=np.float32)
    for c in range(N_CORES):
        b = c // 2
        out[b, qsels[c]] = res.results[c]["out"].astype(np.float32)
    return out


# revision 41
# speedup vs baseline: 1.0001x; 1.0001x over previous
"""Causal self-attention (B=4, T=4096, D=768, single head, fp32 in/out) on 8
TRN2 NeuronCores.

Sharding: core <-> (batch b = core//2, parity h = core%2). Each core handles
the 16 query tiles (128 rows) at global tile index g = 2i + h for local
i = 0..15 (parity interleave balances causal work across the pair to ~3%).
Per local q-tile i the kernel computes scores against keys [0, 256*(i+1)):
columns below 256*i are always causally allowed for both parities; the last
256 columns are fixed up with per-core input mask tiles.

Math restructure vs a direct implementation — no Q/K/V projections at all:
  scores:  S^T = K.Q^T = x_kv . (W_q^T W_k) . x_q^T. M2 = W_q^T W_k is
           computed once (768x768, 11.5us) and R = M2^T x_q^T once
           (768x2048); per kv superblock S^T = x_kv . R uses the streamed
           x^T tile directly as the stationary operand.
  output:  O = P^T V = P^T x_kv W_v^T re-associates to (x_kv^T P^T)^T-style:
           the kernel accumulates G^T[d, q] = sum_k x_kv[k, d] P[k, q]
           across all kv (stationary = x_kv in [kv, d] layout, moving = the
           P^T tile that exp already produces), then applies one output
           transform O[q,:] = sum_d G^T[d, q] W_v^T[d, :] per retired
           q-tile. This replaces the 61us V projection (which was also
           fully duplicated across the core pair) with a 31us transform
           over query columns only.
  softmax: the denominator l[q] = sum_k P[k, q] comes from a per-q-tile
           ones-matmul (K=128, N=1) over a DVE-presummed P (the 4 kv tiles
           of a superblock added elementwise), landing directly in
           [q-partition, 1] form for the final per-partition 1/l scale.
           No max-subtraction pass: scores are ~N(0,1) so exp cannot
           overflow.

All matmul operands are fp16 (PSUM accumulation fp32): fp16 enables the
compiler's fast-weight-load path (fp32 stationary loads at ~190 ns dominate
the PE pipe otherwise) and halves DMA. Output returns as fp16, upcast on
host. Max relative error vs the fp32 reference ~1e-3, vs the 2e-2 gate.

PSUM (8 banks): S^T pool 3 x [128,512] | G^T wave pool 4 x [128,512] |
1 shared bank for l / output-transform. G^T accumulates over a superblock's
4 kv tiles in 3 waves of 2 d-chunks, drained by DVE adds into a [128,
6x2048] fp32 SBUF accumulator.
"""

import os
import sys
from contextlib import ExitStack

import numpy as np

if "/opt/trn_rl_repo" not in sys.path:
    sys.path.insert(0, "/opt/trn_rl_repo")

B, T, D = 4, 4096, 768
N_CORES = 8
QTILES = 16          # local q-tiles per core, 128 rows each
EC = D // 128        # 6 d chunks of 128
SB = 8               # kv superblocks
SBW = 512            # superblock width (keys)
NKT = SBW // 128     # kv 128-tiles per superblock
QW = QTILES * 128    # query columns per core
NEG = -1.0e9
SCALE = 1.0 / float(np.sqrt(D))

_CACHE = {}


def _patch_tile_drain():
    """This walrus build accepts only one sync wait per instruction;
    TileContext's tail drain carries one wait per outstanding proc. Split
    them onto individual SP no-ops (SP executes sequentially, so semantics
    are unchanged)."""
    import concourse.mybir as mybir
    import concourse.tile as tile
    from concourse.vector_clock import ScopedClock

    if getattr(tile.TileContext, "_drain_split_patch", False):
        return

    def _split_drain_and_barrier(self, tick_clock, wait_clock):
        nc = self.nc
        carrier = nc.sync.nop(nofuse=True)
        wait_clock.add_sem_waits(
            carrier.ins, ScopedClock({None: tick_clock.global_clock})
        )
        si = carrier.ins.sync_info
        waits = list(si.on_wait) if si is not None else []
        carrier.ins.sync_info = mybir.SyncInfo(on_wait=waits[:1], on_update=[])
        for w in waits[1:]:
            n = nc.sync.nop(nofuse=True)
            n.ins.sync_info = mybir.SyncInfo(on_wait=[w], on_update=[])
        nc.sync.drain()
        nc.all_engine_barrier()
        assert self.sems is not None
        popped = nc._tile_sem_poison_stack.pop()
        assert popped is self._sem_poison
        nc.clear_and_free_semaphores(list(self.sems.allocated().values()))
        nc.all_engine_barrier()

    tile.TileContext._drain_and_barrier = _split_drain_and_barrier
    tile.TileContext._drain_split_patch = True


def _hoist_multi_waits(nc):
    """This walrus build encodes at most ONE sync wait per instruction
    descriptor. Tile's sem assignment can put several waits on one
    instruction; hoist the extras onto same-engine no-ops inserted
    immediately before it — the engine executes them sequentially, so the
    wait semantics are unchanged."""
    import concourse.mybir as mybir

    n = 0
    for fn in nc.m.functions:
        for bb in fn.blocks:
            insts = bb.instructions
            out = []
            for ins in insts:
                si = ins.sync_info
                waits = list(si.on_wait) if si is not None else []
                if len(waits) > 1:
                    for w in waits[:-1]:
                        nop = mybir.InstNoOp(
                            name=f"I-hoistw-{nc.next_id()}",
                            engine=ins.engine,
                            ins=[],
                            outs=[],
                            sync_info=mybir.SyncInfo(on_wait=[w], on_update=[]),
                        )
                        out.append(nop)
                        n += 1
                    ins.sync_info = mybir.SyncInfo(
                        on_wait=[waits[-1]], on_update=list(si.on_update)
                    )
                out.append(ins)
            insts[:] = out
    return n


def _build_program(hoist=True):
    import concourse.bass as bass
    import concourse.mybir as mybir
    import concourse.tile as tile

    _patch_tile_drain()
    f32 = mybir.dt.float32
    f16 = mybir.dt.float16
    Exp = mybir.ActivationFunctionType.Exp
    Copy = mybir.ActivationFunctionType.Copy

    nc = bass.Bass()
    # x^T per core batch: [d_part, d_chunk, kv] — stationary operand of S^T
    xkvT = nc.dram_tensor("xkvT", [128, EC, T], f16, kind="ExternalInput")
    # x rows per core batch: [kv_part, kv_tile, d] — stationary operand of G^T
    xkvK = nc.dram_tensor("xkvK", [128, SB * NKT, D], f16, kind="ExternalInput")
    xqT = nc.dram_tensor("xqT", [128, EC, QW], f16, kind="ExternalInput")
    # W_q, W_k in stored [e, d] layout chunked over e; W_v^T chunked over d
    wqE = nc.dram_tensor("wqE", [128, EC, D], f16, kind="ExternalInput")
    wkE = nc.dram_tensor("wkE", [128, EC, D], f16, kind="ExternalInput")
    wvT = nc.dram_tensor("wvT", [128, EC, D], f16, kind="ExternalInput")
    # masks are [kv, q] (transposed) here; maskf is all -1e9
    maska = nc.dram_tensor("maska", [128, 128], f32, kind="ExternalInput")
    maskb = nc.dram_tensor("maskb", [128, 128], f32, kind="ExternalInput")
    maskf = nc.dram_tensor("maskf", [128, 128], f32, kind="ExternalInput")
    out_d = nc.dram_tensor("out", [QW, D], f16, kind="ExternalOutput")

    with tile.TileContext(nc) as tc:
        with (
            tc.tile_pool(name="consts", bufs=1) as cpool,
            tc.tile_pool(name="wk", bufs=1) as wkpool,
            tc.tile_pool(name="rt", bufs=1) as rtpool,
            tc.tile_pool(name="gacc", bufs=1) as gapool,
            tc.tile_pool(name="ps_st", bufs=3, space="PSUM") as ps_st,
            tc.tile_pool(name="ps_gt", bufs=4, space="PSUM") as ps_gt,
            tc.tile_pool(name="ps_lo", bufs=1, space="PSUM") as ps_lo,
        ):
            # PE warm-up: the HAM clock gate starts at 1.2 GHz and needs
            # ~3.4us of sustained matmul activity to release to 2.4 GHz.
            # Burn the initial DMA wait on dummy matmuls.
            warm_t = cpool.tile([128, 512], f16, tag="warm")
            nc.vector.memset(warm_t[:], 0.0)
            for w in range(26):
                wps = ps_gt.tile([128, 512], f32, tag="gt", name=f"warm{w}")
                nc.tensor.matmul(
                    wps[:], warm_t[:, :128], warm_t[:], start=True, stop=True
                )

            # x superblock tiles prefetch in dedicated pools (outside the
            # transient qproj space) so sb0/sb1 stream during M2/R.
            xtpool_cm = tc.tile_pool(name="xt", bufs=3)
            xtpool = xtpool_cm.__enter__()
            xkpool_cm = tc.tile_pool(name="xk", bufs=3)
            xkpool = xkpool_cm.__enter__()
            qppool_cm = tc.tile_pool(name="qproj", bufs=1)
            qppool = qppool_cm.__enter__()
            wq_t = qppool.tile([128, EC, D], f16, tag="wq")
            nc.sync.dma_start(out=wq_t[:], in_=wqE[:])
            wk_t = qppool.tile([128, EC, D], f16, tag="wk")
            nc.sync.dma_start(out=wk_t[:], in_=wkE[:])
            xq_t = qppool.tile([128, EC, QW], f16, tag="xq")
            nc.sync.dma_start(out=xq_t[:], in_=xqT[:])
            m2_t = qppool.tile([128, EC, D], f16, tag="m2")
            xT_tiles = {}
            xK_tiles = {}
            for sb in range(2):
                xT_tiles[sb] = xtpool.tile(
                    [128, EC, SBW], f16, tag="xt", name=f"xT{sb}"
                )
                nc.sync.dma_start(
                    out=xT_tiles[sb][:],
                    in_=xkvT[:, :, sb * SBW : (sb + 1) * SBW],
                )
                xK_tiles[sb] = xkpool.tile(
                    [128, NKT, D], f16, tag="xk", name=f"xK{sb}"
                )
                nc.sync.dma_start(
                    out=xK_tiles[sb][:],
                    in_=xkvK[:, sb * NKT : (sb + 1) * NKT, :],
                )
            ma_t = cpool.tile([128, 128], f32, tag="ma")
            nc.sync.dma_start(out=ma_t[:], in_=maska[:])
            mb_t = cpool.tile([128, 128], f32, tag="mb")
            nc.sync.dma_start(out=mb_t[:], in_=maskb[:])
            mf_t = cpool.tile([128, 128], f32, tag="mf")
            nc.sync.dma_start(out=mf_t[:], in_=maskf[:])
            wv_t = wkpool.tile([128, EC, D], f16, tag="wv")
            nc.sync.dma_start(out=wv_t[:], in_=wvT[:])
            ones_t = cpool.tile([128, 1], f16, tag="ones")
            nc.vector.memset(ones_t[:], 1.0)

            # ---- M2 = W_q^T W_k  [d2, d], chunked over d2 ----
            for a in range(EC):
                ps = ps_gt.tile([128, 512], f32, tag="gt", name=f"psm2a{a}")
                ps2 = ps_gt.tile([128, 512], f32, tag="gt", name=f"psm2b{a}")
                for j in range(EC):
                    nc.tensor.matmul(
                        ps[:],
                        wq_t[:, j, a * 128 : (a + 1) * 128],
                        wk_t[:, j, 0:512],
                        start=(j == 0),
                        stop=(j == EC - 1),
                    )
                    nc.tensor.matmul(
                        ps2[:, :256],
                        wq_t[:, j, a * 128 : (a + 1) * 128],
                        wk_t[:, j, 512:768],
                        start=(j == 0),
                        stop=(j == EC - 1),
                    )
                nc.scalar.copy(out=m2_t[:, a, 0:512], in_=ps[:])
                nc.scalar.copy(out=m2_t[:, a, 512:768], in_=ps2[:, :256])

            # ---- R = M2^T x_q^T  [d, q] resident in SBUF ----
            r_t = rtpool.tile([128, EC, QW], f16, tag="rt")
            for m in range(EC):
                pss = [
                    ps_st.tile([128, 512], f32, tag="st", name=f"psr{m}_{qc}")
                    for qc in range(3)
                ] + [ps_gt.tile([128, 512], f32, tag="gt", name=f"psr3_{m}")]
                for j in range(EC):
                    for qc in range(4):
                        nc.tensor.matmul(
                            pss[qc][:],
                            m2_t[:, j, m * 128 : (m + 1) * 128],
                            xq_t[:, j, qc * 512 : (qc + 1) * 512],
                            start=(j == 0),
                            stop=(j == EC - 1),
                        )
                for qc in range(4):
                    nc.scalar.copy(
                        out=r_t[:, m, qc * 512 : (qc + 1) * 512], in_=pss[qc][:]
                    )
            qppool_cm.__exit__(None, None, None)

            # G^T accumulator [d_part, d_chunk, q] fp32 and l accumulator
            gacc_t = gapool.tile([128, EC, QW], f32, tag="gacc")
            lacc_t = gapool.tile([128, QTILES], f32, tag="lacc")

            # ---- kv superblocks ----
            attn_pools = ExitStack()
            ptpool = attn_pools.enter_context(tc.tile_pool(name="pt", bufs=3))
            p4pool = attn_pools.enter_context(tc.tile_pool(name="p4", bufs=2))
            gqpool = attn_pools.enter_context(tc.tile_pool(name="gq", bufs=2))
            spool = attn_pools.enter_context(tc.tile_pool(name="small", bufs=2))
            obpool = attn_pools.enter_context(tc.tile_pool(name="ob", bufs=2))
            for sb in range(SB):
                xT_t = xT_tiles.pop(sb)
                xK_t = xK_tiles.pop(sb)
                if sb + 2 < SB:  # prefetch two superblocks ahead
                    xT_tiles[sb + 2] = xtpool.tile(
                        [128, EC, SBW], f16, tag="xt", name=f"xT{sb + 2}"
                    )
                    nc.sync.dma_start(
                        out=xT_tiles[sb + 2][:],
                        in_=xkvT[:, :, (sb + 2) * SBW : (sb + 3) * SBW],
                    )
                    xK_tiles[sb + 2] = xkpool.tile(
                        [128, NKT, D], f16, tag="xk", name=f"xK{sb + 2}"
                    )
                    nc.sync.dma_start(
                        out=xK_tiles[sb + 2][:],
                        in_=xkvK[:, (sb + 2) * NKT : (sb + 3) * NKT, :],
                    )

                # ---- attention, in q-groups of up to 512 columns ----
                # active q-tiles: i in [2*sb, 16); groups are 512-aligned
                i_lo = 2 * sb
                g_lo = i_lo // 4
                for g in range(g_lo, 4):
                    ia = max(i_lo, 4 * g)      # first active q-tile in group
                    ib = 4 * g + 4             # end q-tile (exclusive)
                    qc0 = ia * 128             # first active q column
                    gw = (ib - ia) * 128       # group width (256 or 512)

                    # S^T = x_kv . R for the group's q span, per kv-tile.
                    # In the sb's first group, q-tile 2sb sits at position
                    # 0 and its kv-tiles 2,3 are fully masked on BOTH
                    # parities (SPMD-safe intersection): skip those 128
                    # columns entirely — their P is memset to 0 instead.
                    narrow = {2: 128, 3: 128} if g == g_lo else {}
                    stg = [
                        ps_st.tile([128, 512], f32, tag="st", name=f"st{sb}_{g}_{k}")
                        for k in range(NKT)
                    ]
                    for kt in range(NKT):
                        nlo = narrow.get(kt, 0)
                        for j in range(EC):
                            nc.tensor.matmul(
                                stg[kt][:, nlo:gw],
                                xT_t[:, j, kt * 128 : (kt + 1) * 128],
                                r_t[:, j, qc0 + nlo : qc0 + gw],
                                start=(j == 0),
                                stop=(j == EC - 1),
                            )
                    # causal fixups for the terminal q-tiles of this sb:
                    # q-tile 2sb terminates at kv-tiles (0,1) of this sb
                    # (mask a,b; tiles 2,3 fully masked); q-tile 2sb+1
                    # terminates at kv-tiles (2,3).
                    for i, kts in ((2 * sb, (ma_t, mb_t, mf_t, mf_t)),
                                   (2 * sb + 1, (None, None, ma_t, mb_t))):
                        if not (ia <= i < ib):
                            continue
                        qo = i * 128 - qc0
                        for kt in range(NKT):
                            m = kts[kt]
                            if m is not None and qo >= narrow.get(kt, 0):
                                nc.vector.tensor_add(
                                    stg[kt][:, qo : qo + 128],
                                    stg[kt][:, qo : qo + 128],
                                    m[:],
                                )
                    # P^T = exp(S^T * scale) back to SBUF (fp16); narrowed
                    # columns never touch PSUM — memset their P to 0.
                    pt_t = ptpool.tile([128, NKT, 512], f16, tag="pt")
                    for kt in range(NKT):
                        nlo = narrow.get(kt, 0)
                        if nlo:
                            nc.vector.memset(pt_t[:, kt, :nlo], 0.0)
                        nc.scalar.activation(
                            pt_t[:, kt, nlo:gw], stg[kt][:, nlo:gw], Exp, scale=SCALE
                        )
                    # l path: DVE-presum the 4 P^T kv tiles elementwise
                    # (fp16, on the slack Vector engine); then ONE
                    # K=128/N=1 ones-matmul per active q-tile lands the
                    # denominator directly in [q_part, 1] form.
                    p4_t = p4pool.tile([128, 512], f16, tag="p4")
                    nc.vector.tensor_add(
                        p4_t[:, :gw], pt_t[:, 0, :gw], pt_t[:, 1, :gw]
                    )
                    nc.vector.tensor_add(
                        p4_t[:, :gw], p4_t[:, :gw], pt_t[:, 2, :gw]
                    )
                    nc.vector.tensor_add(
                        p4_t[:, :gw], p4_t[:, :gw], pt_t[:, 3, :gw]
                    )
                    # G^T += x_kv^T . P^T in 3 waves of 2 d-chunks. The l
                    # ones-matmuls (array-cheap but LDW-serial in the
                    # in-order PE queue) interleave between waves so their
                    # weight loads hide under the waves' streaming.
                    lq = list(range(ia, ib))
                    for wv in range(3):
                        gps = [
                            ps_gt.tile(
                                [128, 512], f32, tag="gt",
                                name=f"gt{sb}_{g}_{wv}_{c}",
                            )
                            for c in range(2)
                        ]
                        for kt in range(NKT):
                            nlo = narrow.get(kt, 0)
                            for c in range(2):
                                mch = 2 * wv + c
                                nc.tensor.matmul(
                                    gps[c][:, nlo:gw],
                                    xK_t[:, kt, mch * 128 : (mch + 1) * 128],
                                    pt_t[:, kt, nlo:gw],
                                    start=(kt == 0),
                                    stop=(kt == NKT - 1),
                                )
                        for _ in range(2 if wv == 2 else 1):
                            if not lq:
                                continue
                            i = lq.pop(0)
                            qo = i * 128 - qc0
                            lps = ps_lo.tile(
                                [128, 512], f32, tag="lo", name=f"l{sb}_{i}"
                            )
                            nc.tensor.matmul(
                                lps[:, 0:1],
                                p4_t[:, qo : qo + 128],
                                ones_t[:],
                                start=True,
                                stop=True,
                            )
                            lsl = lacc_t[:, i : i + 1]
                            if sb == 0:
                                nc.vector.tensor_copy(out=lsl, in_=lps[:, 0:1])
                            else:
                                nc.vector.tensor_add(lsl, lps[:, 0:1], lsl)
                        for c in range(2):
                            mch = 2 * wv + c
                            gsl = gacc_t[:, mch, qc0 : qc0 + gw]
                            if sb == 0:
                                nc.vector.tensor_copy(out=gsl, in_=gps[c][:, :gw])
                            else:
                                nc.vector.tensor_add(gsl, gps[c][:, :gw], gsl)

                    # ---- retire terminal q-tiles: O = G^T^T . W_v^T ----
                    for i in range(ia, ib):
                        if i not in (2 * sb, 2 * sb + 1):
                            continue
                        # convert this q-tile's G^T slice to fp16 stationary
                        gq_t = gqpool.tile([128, EC, 128], f16, tag="gq")
                        nc.scalar.copy(
                            out=gq_t[:], in_=gacc_t[:, :, i * 128 : (i + 1) * 128]
                        )
                        recip = spool.tile([128, 1], f32, tag="recip")
                        nc.vector.reciprocal(out=recip[:], in_=lacc_t[:, i : i + 1])
                        ob = obpool.tile([128, D], f16, tag="ob")
                        for lo, n in ((0, 512), (512, 256)):
                            # second half borrows a Gt-pool bank so the two
                            # matmul groups + normalizes pipeline
                            pool = ps_lo if lo == 0 else ps_gt
                            ops = pool.tile(
                                [128, 512], f32,
                                tag=("lo" if lo == 0 else "gt"),
                                name=f"o{sb}_{i}_{lo}",
                            )
                            for m in range(EC):
                                nc.tensor.matmul(
                                    ops[:, :n],
                                    gq_t[:, m, :],
                                    wv_t[:, m, lo : lo + n],
                                    start=(m == 0),
                                    stop=(m == EC - 1),
                                )
                            nc.scalar.activation(
                                ob[:, lo : lo + n],
                                ops[:, :n],
                                Copy,
                                scale=recip[:, 0:1],
                            )
                        nc.sync.dma_start(
                            out=out_d[i * 128 : (i + 1) * 128, :], in_=ob[:]
                        )
            attn_pools.close()
            xkpool_cm.__exit__(None, None, None)
            xtpool_cm.__exit__(None, None, None)
    if hoist:
        _hoist_multi_waits(nc)
    return nc


def _prep_inputs(x, W_q, W_k, W_v):
    """Per-core input maps. Host-side work is layout + fp16 cast only."""

    def chunked(a):  # [768, N] -> [128, EC, N]
        return np.ascontiguousarray(
            a.reshape(EC, 128, -1).transpose(1, 0, 2).astype(np.float16)
        )

    wqE = chunked(W_q)           # [e, d] chunked over e
    wkE = chunked(W_k)
    wvT = chunked(W_v.T.copy())  # [d, e] chunked over d

    r = np.arange(128, dtype=np.float32)
    # [q, c] triangle: allowed iff c <= q; stored transposed ([kv, q])
    tri = np.where(r[None, :] <= r[:, None], 0.0, NEG).astype(np.float32)
    triT = np.ascontiguousarray(tri.T)
    zero = np.zeros((128, 128), dtype=np.float32)
    full = np.full((128, 128), NEG, dtype=np.float32)
    # per-parity (maska, maskb) for the terminal 256 kv columns
    masks_ab = [(triT, full), (zero, triT)]

    in_maps = []
    qsels = []
    for c in range(N_CORES):
        b, h = c // 2, c % 2
        xT = chunked(np.ascontiguousarray(x[b].T))  # [128, EC, T] fp16
        # [kv_part, kv_tile, d]: row sb*512 + kt*128 + p -> [p, sb*4+kt, :]
        xK = np.ascontiguousarray(
            x[b].reshape(SB * NKT, 128, D).transpose(1, 0, 2).astype(np.float16)
        )
        qsel = np.concatenate(
            [np.arange((2 * i + h) * 128, (2 * i + h + 1) * 128) for i in range(QTILES)]
        )
        qsels.append(qsel)
        ma, mb = masks_ab[h]
        in_maps.append(
            {
                "xkvT": xT,
                "xkvK": xK,
                "xqT": np.ascontiguousarray(xT[:, :, qsel]),
                "wqE": wqE,
                "wkE": wkE,
                "wvT": wvT,
                "maska": ma,
                "maskb": mb,
                "maskf": full,
            }
        )
    return in_maps, qsels


def kernel(x, W_q, W_k, W_v, _trace=False):
    from concourse.bass_utils import run_bass_kernel_spmd

    if "nc" not in _CACHE:
        _CACHE["nc"] = _build_program()
    nc = _CACHE["nc"]

    in_maps, qsels = _prep_inputs(
        np.asarray(x, dtype=np.float32),
        np.asarray(W_q, dtype=np.float32),
        np.asarray(W_k, dtype=np.float32),
        np.asarray(W_v, dtype=np.float32),
    )
    res = run_bass_kernel_spmd(nc, in_maps, list(range(N_CORES)), trace=_trace)
    _CACHE["last_results"] = res

    out = np.empty((B, T, D), dtype=np.float32)
    for c in range(N_CORES):
        b = c // 2
        out[b, qsels[c]] = res.results[c]["out"].astype(np.float32)
    return out
